# revision 59
# baseline (speedup 1.0000x reference)
"""Trainium2 Bass kernel for nn_Attention_sep (separate patch/det QKV attention).

Sharding: query rows split across 8 cores (528 patch + 16 det queries per
core, zero-padded); K/V projections replicated per core (each core needs all
4301 keys — a cross-core exchange is slower than recompute under the D2D /
collective cost models). Everything flows in bf16 (1 PE cycle/row, half the
DMA and SBUF of fp32), and K^T / V live in per-x-chunk SBUF-resident tiles
instead of round-tripping through DRAM scratch. The K/V projection is emitted
as a stream of small units interleaved into the attention instruction slots,
so TensorE fills its exp-wait gaps with projection matmuls for the next
superblock group. Attention runs q-block-outer, keys-major: S^T = K_h^T'Q_h^T
per 128-key chunk (head pairs at partition bases 0/64), exp(SCALE*s) on
ScalarE straight out of PSUM into bf16, then attn@V accumulates o^T (+ sumexp
via a ones column in V) in two PSUM banks across 8-chunk groups before a
single DVE flush-add. The qb0 LayerNorm/out-proj tail overlaps qb1's
ACT-bound attention; the tail transposes heads to token-major (bf16
transposes), divides by sumexp, applies LayerNorm (bn_stats/bn_aggr, exact
eps), and transposes back for the bf16 output projection.

Host only slices/transposes/casts inputs and gathers per-core outputs.
Dispatch uploads shared inputs sharded (1x wire) and replicates them
on-device; replicated weights are cached across calls.
"""
import sys
sys.path.insert(0, "/opt/trn_rl_repo")
import numpy as np
from ml_dtypes import bfloat16

N_TOK = 4301
D = 768
H = 12
HD = 64
NDET = 100
NPATCH = N_TOK - NDET          # 4201
SCALE = HD ** -0.5
EPS = 1e-5
NCORES = 8
PQ = 528                        # per-core patch queries (528*8 = 4224 >= 4201)
DQ = 16                         # per-core det queries (16*8 = 128 >= 100)
TQ = PQ + DQ                    # 544
QB = TQ // 2                    # 272 (one PSUM bank per q-block)
DC = D // 128                   # 6 feature/contraction chunks

# key chunks: 32 x 128 patch, 105 patch tail, 100 det  (exactly 4301 keys)
KC_SIZES = [128] * 32 + [105, 100]
KC_STARTS = [128 * i for i in range(32)] + [4096, 4201]
NKC = len(KC_SIZES)             # 34
KB = 4                          # key chunks per K/V superblock (= one x-chunk)
# x token chunks for the projection phase (aligned with key chunks)
XN_SIZES = [512] * 8 + [105, 100]
XN_STARTS = [512 * i for i in range(8)] + [4096, 4201]

_CACHE = {}


def _build(phases=3):
    import concourse.bass as bass
    import concourse.tile as tile
    from concourse import bacc, mybir
    from concourse.masks import make_identity

    FP32 = mybir.dt.float32
    BF16 = mybir.dt.bfloat16
    AF = mybir.ActivationFunctionType
    ALU = mybir.AluOpType

    nc = bacc.Bacc(name="attn_sep")

    def din(name, shape, dt=FP32):
        return nc.dram_tensor(name, shape, dt, kind="ExternalInput")

    xT = din("xT", [D, N_TOK], BF16)
    xqT = din("xqT", [D, TQ], BF16)
    w_in = {k: din(k, [D, D], BF16) for k in
            ["wqT_p", "wqT_d", "wkT_p", "wkT_d", "wvT_p", "wvT_d",
             "woT_p", "woT_d"]}
    b_in = {k: din(k, [D]) for k in
            ["bq_p", "bq_d", "bv_p", "bv_d", "bo_p", "bo_d", "ln_g", "ln_b"]}
    b16_in = {k: din(k + "16", [D], BF16)
              for k in ["ln_g", "ln_b", "bv_p", "bv_d"]}
    outT = nc.dram_tensor("outT", [D, TQ], FP32, kind="ExternalOutput")
    outT_v = outT.rearrange("(c p) q -> p c q", p=128)
    xT_v = xT.rearrange("(c p) n -> p c n", p=128)
    xqT_v = xqT.rearrange("(c p) n -> p c n", p=128)

    from contextlib import ExitStack
    with tile.TileContext(nc) as tc:
        with ExitStack() as ctx:
            ep = ctx.enter_context
            qtp = ep(tc.tile_pool(name="qtp", bufs=1))
            wp = ep(tc.tile_pool(name="wp", bufs=2))
            xp = ep(tc.tile_pool(name="xp", bufs=2))
            kvp = ep(tc.tile_pool(name="kvp", bufs=1))
            ptp = ep(tc.tile_pool(name="ptp", bufs=3))
            oap = ep(tc.tile_pool(name="oap", bufs=2))
            asp = ep(tc.tile_pool(name="asp", bufs=3))
            onp = ep(tc.tile_pool(name="onp", bufs=3))
            olp = ep(tc.tile_pool(name="olp", bufs=1))
            oup = ep(tc.tile_pool(name="oup", bufs=2))
            sgl = ep(tc.tile_pool(name="sgl", bufs=1))
            sml = ep(tc.tile_pool(name="sml", bufs=8))
            pjb = ep(tc.tile_pool(name="pjb", bufs=2, space="PSUM"))
            pob = ep(tc.tile_pool(name="pob", bufs=2, space="PSUM"))
            pss = ep(tc.tile_pool(name="pss", bufs=2, space="PSUM"))
            # ---- constants / broadcast tiles ----
            ident = sgl.tile([128, 128], BF16, tag="ident")
            make_identity(nc, ident)

            def bcast(name, dt=FP32, src_map=None):
                t = sgl.tile([128, D], dt, tag=f"bc_{name}")
                src = (src_map or b_in)[name][:]
                nc.gpsimd.dma_start(
                    out=t,
                    in_=bass.AP(tensor=src.tensor, offset=src.offset,
                                ap=[[0, 128]] + [list(a) for a in src.ap]))
                return t

            bv_p_b = bcast("bv_p", BF16, b16_in)
            bv_d_b = bcast("bv_d", BF16, b16_in)
            g_b = bcast("ln_g", BF16, b16_in)
            b_b = bcast("ln_b", BF16, b16_in)

            def perpart(name):
                t = sgl.tile([128, DC], FP32, tag=f"pp_{name}")
                nc.sync.dma_start(t, b_in[name].rearrange("(c p) -> p c", p=128))
                return t

            eps_t = sgl.tile([128, 1], FP32, tag="eps")
            nc.vector.memset(eps_t, EPS)
            bq_p_s = perpart("bq_p")
            bq_d_s = perpart("bq_d")
            bo_p_s = perpart("bo_p")
            bo_d_s = perpart("bo_d")

            def load_w(name):
                t = wp.tile([128, DC, D], BF16, tag="w")
                nc.sync.dma_start(t, w_in[name].rearrange("(c p) f -> p c f", p=128))
                return t

            # ---- resident tensors ----
            QT = qtp.tile([128, DC, TQ], BF16, tag="QT")
            # per-x-chunk SBUF-resident K^T (feature-major) and V (token-major
            # with a ones column per head); separate tiles per chunk so the
            # attention superblock for chunk j only waits on chunk j's writes
            Ksb = [kvp.tile([128, DC, XN_SIZES[j]], BF16, tag=f"ks{j}",
                            name=f"ks{j}")
                   for j in range(len(XN_SIZES))]
            Vsb = [kvp.tile([128, (XN_SIZES[j] + 127) // 128, H, HD + 1],
                            BF16, tag=f"vs{j}", name=f"vs{j}")
                   for j in range(len(XN_SIZES))]
            for j in range(len(XN_SIZES)):
                nc.gpsimd.memset(Vsb[j][:, :, :, HD:HD + 1], 1.0)

            # =========== Q^T projection (feature-major), bias added ===========
            wq_p = load_w("wqT_p")
            wq_d = load_w("wqT_d")
            xq = xp.tile([128, DC, TQ], BF16, tag="xq")
            nc.sync.dma_start(xq, xqT_v)
            q_segs = [(0, QB, wq_p, bq_p_s), (QB, PQ - QB, wq_p, bq_p_s),
                      (PQ, DQ, wq_d, bq_d_s)]
            for fc in range(DC):
                for c0, n, wq, bq in q_segs:
                    pq = pjb.tile([128, 512], FP32, tag="bank")
                    for dc in range(DC):
                        nc.tensor.matmul(
                            pq[:, :n],
                            wq[:, dc, 128 * fc:128 * (fc + 1)],
                            xq[:, dc, c0:c0 + n],
                            start=(dc == 0), stop=(dc == DC - 1))
                    nc.vector.tensor_scalar_add(
                        QT[:, fc, c0:c0 + n], pq[:, :n], bq[:, fc:fc + 1])

            # ====== K^T/V projection as a unit stream, interleaved with
            # ====== attention so TensorE fills its exp-wait gaps with
            # ====== projection matmuls for the NEXT superblock group
            st = {"wk": load_w("wkT_p"), "wv": load_w("wvT_p"),
                  "bvb": bv_p_b, "xt": None}

            def proj_chunk_units(nch):
                n0, sz = XN_STARTS[nch], XN_SIZES[nch]

                def u_det():
                    st["wk"] = load_w("wkT_d")
                    st["wv"] = load_w("wvT_d")
                    st["bvb"] = bv_d_b

                def u_x():
                    xt = xp.tile([128, DC, 512], BF16, tag="x")
                    nc.sync.dma_start(xt[:, :, :sz], xT_v[:, :, n0:n0 + sz])
                    st["xt"] = xt

                def mk_k(fc):
                    def u():
                        pk = pjb.tile([128, 512], FP32, tag="bank")
                        for dc in range(DC):
                            nc.tensor.matmul(
                                pk[:, :sz],
                                st["wk"][:, dc, 128 * fc:128 * (fc + 1)],
                                st["xt"][:, dc, :sz],
                                start=(dc == 0), stop=(dc == DC - 1))
                        nc.vector.tensor_copy(Ksb[nch][:, fc, :sz], pk[:, :sz])
                    return u

                def mk_v(s0, half):
                    m = min(128, sz - s0)

                    def u():
                        pv = pjb.tile([128, 512], FP32, tag="bank")
                        f0 = half * 384
                        for dc in range(DC):
                            nc.tensor.matmul(
                                pv[:m, :384],
                                st["xt"][:, dc, s0:s0 + m],
                                st["wv"][:, dc, f0:f0 + 384],
                                start=(dc == 0), stop=(dc == DC - 1))
                        nc.vector.tensor_tensor(
                            Vsb[nch][:m, s0 // 128, 6 * half:6 * (half + 1), :HD],
                            pv[:m, :384].rearrange("p (h d) -> p h d", d=HD),
                            st["bvb"][:m, f0:f0 + 384].rearrange(
                                "p (h d) -> p h d", d=HD),
                            ALU.add)
                    return u

                units = ([u_det] if nch == 9 else []) + [u_x]
                units += [mk_k(fc) for fc in range(DC)]
                units += [mk_v(s0, half) for s0 in range(0, sz, 128)
                          for half in range(2)]
                return units

            UNITS = []
            UPTO = {}           # group -> units that must be emitted first
            for g, grp in enumerate([(0, 1), (2, 3), (4, 5), (6, 7), (8, 9)]):
                for nch in grp:
                    UNITS += proj_chunk_units(nch)
                UPTO[g] = len(UNITS)
            emitted = [0]

            def drain(upto):
                while emitted[0] < upto:
                    UNITS[emitted[0]]()
                    emitted[0] += 1

            # ====== attention: q-block outer, superblock groups inner ======
            # Per (qb, group, pj): o^T accumulates in 2 PSUM banks over the
            # group's 8 key chunks, then one DVE flush-add per par. Projection
            # units for group g+1 drain into the exp-wait slots of group g.
            # The qb0 LN/out-proj tail runs while qb1's attention keeps
            # ScalarE busy.
            oaccs = [oap.tile([65, H, QB], BF16, tag="oacc", name=f"oacc{qb}")
                     for qb in range(2)]
            SB_GROUPS = [(0, 1), (2, 3), (4, 5), (6, 7), (8, 9)] \
                if phases >= 2 else []
            for qb in range(2):
                q0 = qb * QB
                for g, grp in enumerate(SB_GROUPS):
                    drain(UPTO[g])      # group's own K/V must be complete
                    gchunks = []
                    for sbj in grp:
                        cs = (list(range(4 * sbj, 4 * sbj + 4)) if sbj < 8
                              else [32 + (sbj - 8)])
                        gchunks += [(sbj, ch) for ch in cs]
                    nch = len(gchunks)
                    for pj in range(DC):
                        po = [pob.tile([65, QB], FP32, tag="bank",
                                       name=f"po{par}")
                              for par in range(2)]
                        for ci0, (sbj, ch) in enumerate(gchunks):
                            kt, vs = Ksb[sbj], Vsb[sbj]
                            lk0 = KC_STARTS[ch] - XN_STARTS[sbj]
                            ci = lk0 // 128
                            kc = KC_SIZES[ch]
                            ps = pss.tile([128, 2, 512], FP32, tag="s2")
                            for par in range(2):
                                pb = 64 * par
                                nc.tensor.matmul(
                                    ps[:kc, par, :QB],
                                    kt[pb:pb + 64, pj, lk0:lk0 + kc],
                                    QT[pb:pb + 64, pj, q0:q0 + QB],
                                    start=True, stop=True)
                            pt = ptp.tile([128, 2, QB], BF16, tag="pt")
                            nc.scalar.activation(
                                pt[:kc], ps[:kc, :, :QB], AF.Exp, scale=SCALE)
                            for par in range(2):
                                h = 2 * pj + par
                                nc.tensor.matmul(
                                    po[par],
                                    vs[:kc, ci, h, :],
                                    pt[:kc, par, :],
                                    start=(ci0 == 0),
                                    stop=(ci0 == nch - 1))
                            if qb == 0 and g + 1 in UPTO:
                                # fill the exp-wait gap with one projection
                                # unit for the next group
                                if emitted[0] < UPTO[g + 1]:
                                    UNITS[emitted[0]]()
                                    emitted[0] += 1
                        for par in range(2):
                            h = 2 * pj + par
                            if g == 0:
                                nc.vector.tensor_copy(
                                    oaccs[qb][:, h, :], po[par])
                            else:
                                nc.vector.tensor_add(
                                    oaccs[qb][:, h, :], oaccs[qb][:, h, :],
                                    po[par])

                # ======= LN + out-proj tail for this q-block =======
                if phases < 3:
                    continue
                if "wo_p" not in st:
                    st["wo_p"] = load_w("woT_p")
                    st["wo_d"] = load_w("woT_d")
                wo_p, wo_d = st["wo_p"], st["wo_d"]
                oacc = oaccs[qb]
                # ---- tail: transpose heads, divide, LayerNorm, transpose ----
                o_lnT = olp.tile([128, DC, QB], BF16, tag="olnT")
                for off, L in [(0, 128), (128, 128), (256, QB - 256)]:
                    o_asm = asp.tile([128, H, HD + 1], BF16, tag="oasm")
                    for h in range(H):
                        tp = pjb.tile([128, 512], BF16, tag="bank")
                        nc.tensor.transpose(
                            tp[:L, :65], oacc[:, h, off:off + L], ident[:65, :65])
                        # the final tail runs after the last exp, so ScalarE
                        # is idle there and can drain the transposes instead
                        # of lengthening the DVE chain; the qb0 tail overlaps
                        # qb1's exp stream, so it must stay off ScalarE
                        if qb == 1:
                            nc.scalar.copy(o_asm[:L, h, :], tp[:L, :65])
                        else:
                            nc.vector.tensor_copy(o_asm[:L, h, :], tp[:L, :65])
                    rs = sml.tile([128, H], FP32, tag="rs")
                    nc.vector.reciprocal(rs[:L], o_asm[:L, :, HD])
                    o_n = onp.tile([128, D], BF16, tag="on")
                    for h in range(H):
                        nc.vector.tensor_scalar_mul(
                            o_n[:L, HD * h:HD * (h + 1)],
                            o_asm[:L, h, :HD], rs[:L, h:h + 1])
                    stats = sml.tile([128, 3, 6], FP32, tag="st")
                    for gi in range(3):
                        nc.vector.bn_stats(
                            stats[:L, gi], o_n[:L, 256 * gi:256 * (gi + 1)])
                    mv = sml.tile([128, 2], FP32, tag="mv")
                    nc.vector.bn_aggr(mv[:L], stats[:L])
                    rstd = sml.tile([128, 1], FP32, tag="rstd")
                    nc.scalar.activation(rstd[:L], mv[:L, 1:2], AF.Sqrt,
                                         bias=eps_t[:L])
                    nc.vector.reciprocal(rstd[:L], rstd[:L])
                    nc.vector.tensor_scalar(
                        o_n[:L], o_n[:L], mv[:L, 0:1], rstd[:L],
                        ALU.subtract, ALU.mult)
                    nc.vector.tensor_tensor(o_n[:L], o_n[:L], g_b[:L], ALU.mult)
                    nc.vector.tensor_tensor(o_n[:L], o_n[:L], b_b[:L], ALU.add)
                    for fc in range(DC):
                        tp = pjb.tile([128, 512], BF16, tag="bank")
                        nc.tensor.transpose(
                            tp[:, :L], o_n[:L, 128 * fc:128 * (fc + 1)],
                            ident[:L, :L])
                        if qb == 1:
                            nc.scalar.copy(
                                o_lnT[:, fc, off:off + L], tp[:, :L])
                        else:
                            nc.vector.tensor_copy(
                                o_lnT[:, fc, off:off + L], tp[:, :L])

                # ---- output projection for this q-block ----
                if qb == 0:
                    segs = [(0, QB, wo_p, bo_p_s)]
                else:
                    segs = [(QB, PQ - QB, wo_p, bo_p_s), (PQ, DQ, wo_d, bo_d_s)]
                for fc in range(DC):
                    for c0, n, wo, bo in segs:
                        pu = pjb.tile([128, 512], FP32, tag="bank")
                        for dc in range(DC):
                            nc.tensor.matmul(
                                pu[:, :n],
                                wo[:, dc, 128 * fc:128 * (fc + 1)],
                                o_lnT[:, dc, c0 - q0:c0 - q0 + n],
                                start=(dc == 0), stop=(dc == DC - 1))
                        ou = oup.tile([128, 512], FP32, tag="ou")
                        nc.vector.tensor_scalar_add(
                            ou[:, :n], pu[:, :n], bo[:, fc:fc + 1])
                        nc.sync.dma_start(outT_v[:, fc, c0:c0 + n], ou[:, :n])

    nc.compile()
    return nc


def _run_spmd_dedup(nc, shared, percore):
    """Dispatch the prebuilt Bass module on 8 cores via PJRT.

    Shared inputs are uploaded sharded (1x wire traffic) and replicated
    on-device; donated output buffers are created on-device. Device-resident
    replicas are cached by content hash across calls."""
    import zlib
    import jax
    import jax.numpy as jnp
    from jax.experimental.shard_map import shard_map
    from jax.sharding import Mesh, PartitionSpec as P, NamedSharding
    from concourse import bass2jax, mybir

    bass2jax.install_neuronx_cc_hook()
    partition_name = (nc.partition_id_tensor.name
                      if nc.partition_id_tensor else None)
    in_names, out_names, out_avals = [], [], []
    for alloc in nc.m.functions[0].allocations:
        if not isinstance(alloc, mybir.MemoryLocationSet):
            continue
        name = alloc.memorylocations[0].name
        if alloc.kind == "ExternalInput":
            if name != partition_name:
                in_names.append(name)
        elif alloc.kind == "ExternalOutput":
            out_names.append(name)
            shape = tuple(alloc.tensor_shape)
            out_avals.append(jax.core.ShapedArray(shape, mybir.dt.np(alloc.dtype)))
    n_params = len(in_names)
    all_names = in_names + out_names
    if partition_name is not None:
        all_names = all_names + [partition_name]

    def _body(*args):
        ops = list(args)
        if partition_name is not None:
            ops.append(bass2jax.partition_id_tensor())
        outs = bass2jax._bass_exec_p.bind(
            *ops, out_avals=tuple(out_avals), in_names=tuple(all_names),
            out_names=tuple(out_names), lowering_input_output_aliases=(),
            sim_require_finite=True, sim_require_nnan=True, nc=nc)
        return tuple(outs)

    devices = jax.devices()[:NCORES]
    mesh = Mesh(np.asarray(devices), ("core",))
    rep = NamedSharding(mesh, P(None))
    shd = NamedSharding(mesh, P("core"))
    in_specs = tuple(P(None) if n in shared else P("core") for n in in_names) \
        + (P("core"),) * len(out_names)
    out_specs = (P("core"),) * len(out_names)
    donate = tuple(range(n_params, n_params + len(out_names)))
    if "jit_fn" not in _CACHE:
        _CACHE["jit_fn"] = jax.jit(
            shard_map(_body, mesh=mesh, in_specs=in_specs,
                      out_specs=out_specs, check_rep=False),
            donate_argnums=donate, keep_unused=True)
        _CACHE["replicate"] = jax.jit(lambda a: a, out_shardings=rep)
        _CACHE["dev_cache"] = {}

    def dev_shared(name, arr):
        key = (name, arr.shape, zlib.adler32(arr.tobytes()))
        c = _CACHE["dev_cache"]
        if c.get(name, (None, None))[0] == key:
            return c[name][1]
        a_sh = jax.device_put(arr, shd)        # 1x wire traffic
        a_rep = _CACHE["replicate"](a_sh)      # on-device all-gather
        c[name] = (key, a_rep)
        return a_rep

    zeros_fn = _CACHE.setdefault("zeros_fn", jax.jit(
        lambda: tuple(jnp.zeros((NCORES * a.shape[0], *a.shape[1:]), a.dtype)
                      for a in out_avals),
        out_shardings=tuple(shd for _ in out_avals)))

    ins = [dev_shared(n, shared[n]) if n in shared else
           jax.device_put(np.concatenate(percore[n], axis=0), shd)
           for n in in_names]
    zouts = zeros_fn()
    out_arrs = _CACHE["jit_fn"](*ins, *zouts)
    return [
        {name: np.asarray(out_arrs[i]).reshape(NCORES, *out_avals[i].shape)[c]
         for i, name in enumerate(out_names)}
        for c in range(NCORES)
    ]


def kernel(**inputs):
    from concourse import bass_utils

    if "nc" not in _CACHE:
        _CACHE["nc"] = _build()
    nc = _CACHE["nc"]

    f = {k: np.ascontiguousarray(np.asarray(v, dtype=np.float32))
         for k, v in inputs.items()}
    x = f["x"][0]                                   # [4301, 768]
    xT = np.ascontiguousarray(x.T)                  # [768, 4301]

    def bf(a):
        return np.ascontiguousarray(np.asarray(a, dtype=bfloat16))

    base = {
        "xT": bf(xT),
        "wqT_p": bf(f["wq_p"].T), "wqT_d": bf(f["wq_d"].T),
        "wkT_p": bf(f["wk_p"].T), "wkT_d": bf(f["wk_d"].T),
        "wvT_p": bf(f["wv_p"].T), "wvT_d": bf(f["wv_d"].T),
        "woT_p": bf(f["wo_p"].T), "woT_d": bf(f["wo_d"].T),
        "bq_p": f["bq_p"], "bq_d": f["bq_d"],
        "bv_p": f["bv_p"], "bv_d": f["bv_d"],
        "bo_p": f["bo_p"], "bo_d": f["bo_d"],
        "ln_g": f["ln_g"], "ln_b": f["ln_b"],
        "ln_g16": bf(f["ln_g"]), "ln_b16": bf(f["ln_b"]),
        "bv_p16": bf(f["bv_p"]), "bv_d16": bf(f["bv_d"]),
    }
    in_maps = []
    for c in range(NCORES):
        xqT = np.zeros((D, TQ), np.float32)
        p0, p1 = PQ * c, min(PQ * (c + 1), NPATCH)
        if p1 > p0:
            xqT[:, :p1 - p0] = xT[:, p0:p1]
        d0, d1 = DQ * c, min(DQ * (c + 1), NDET)
        if d1 > d0:
            xqT[:, PQ:PQ + d1 - d0] = xT[:, NPATCH + d0:NPATCH + d1]
        in_maps.append({**base, "xqT": bf(xqT)})

    try:
        results = _run_spmd_dedup(
            nc, shared=base,
            percore={"xqT": [m["xqT"] for m in in_maps]})
    except Exception:
        _CACHE.pop("jit_fn", None)
        results = bass_utils.run_bass_kernel_spmd(
            nc, in_maps, core_ids=list(range(NCORES))).results

    out = np.empty((N_TOK, D), np.float32)
    for c in range(NCORES):
        oc = results[c]["outT"].T                   # [544, 768]
        p0, p1 = PQ * c, min(PQ * (c + 1), NPATCH)
        if p1 > p0:
            out[p0:p1] = oc[:p1 - p0]
        d0, d1 = DQ * c, min(DQ * (c + 1), NDET)
        if d1 > d0:
            out[NPATCH + d0:NPATCH + d1] = oc[PQ:PQ + d1 - d0]
    return out[None]
